# revision 23
# baseline (speedup 1.0000x reference)
"""Bilinear RoI pooling (7x7) on 8 Trainium2 NeuronCores.

Strategy (data-parallel over RoIs, per the sharding hint):
  - B=1024 boxes split into 8 slices of 128; the feature map is replicated.
  - Host builds a "quad layout" map: Q[iy, ix] = the 2x2 corner block
    [F[y,x], F[y,x+1], F[y+1,x], F[y+1,x+1]] stored contiguously (4*C
    values), over a zero-padded canvas (2 pad rows/cols on every side).
    One indirect DMA descriptor per (box, grid-point) fetches all 4
    bilinear corners.
  - feat="int8": the map is linearly quantized (global scale s = max|f|/127)
    and the SWDGE gather casts int8 -> bf16 in-flight, halving HBM gather
    traffic; s is folded into the blend weights on device.
  - Corner indices are clamped to [-2, 128] so every out-of-bounds corner
    lands in an all-zero quad (the reference zeroes OOB contributions).
  - blend="pe": the 4-corner weighted sum runs on the tensor engine as 4
    accumulating matmuls whose stationary operand is diag(w_j) (built per
    point by eye*w on the vector engine); PSUM accumulates in f32 and the
    scalar engine copies PSUM -> bf16 SBUF.  The vector engine only does
    index/weight math; stores are grouped and the host upcasts to f32.

Device layout: partition = box (128/core); 49 grid points along free dim.
"""

import numpy as np
import ml_dtypes

P = 128          # boxes per core == SBUF partitions
C = 512          # channels
NPT = 49         # 7*7 grid points
HP = WP = 132    # padded canvas (2 zero rows/cols each side)
NQ = 131         # quad map is NQ x NQ cells of 4*C values
NROW = NQ * NQ
NCORES = 8
MAGIC = 12582912.0  # 1.5*2^23: x+MAGIC stays in [2^23,2^24) where ulp == 1

FEAT = "int8"    # default build config (test.py benches the same config)
BLEND = "pe"
OSTORE = "int8"  # output store dtype; int8 is dequantized host-side by the same s

_STATE = {}


# NOTE: multi-index offset APs pass CoreSim but produce garbage on real
# hardware — the HW indirect DMA only honors [P,1] offsets.
def _build_nc(repeats=1, feat=None, blend=None, ostore=None, bufs=16, abufs=3,
              dbufs=12, pbufs=6, store_group=7, skip_blend=False,
              skip_gather=False, diag_once=False, one_mm=False, pb=1, dvm=0,
              half_gather=False, act2=False):
    import concourse.bass as bass
    import concourse.bacc as bacc
    import concourse.tile as tile
    from concourse import mybir

    feat = feat or FEAT
    blend = blend or BLEND
    ostore = ostore or OSTORE
    assert ostore == "bf16" or (feat == "int8" and blend == "pe")

    F32 = mybir.dt.float32
    BF16 = mybir.dt.bfloat16
    I8 = mybir.dt.int8
    I32 = mybir.dt.int32
    Alu = mybir.AluOpType

    nc = bacc.Bacc()
    qdt = I8 if feat == "int8" else BF16
    qmap = nc.declare_dram_parameter("qmap", [NROW, 4 * C], qdt, isOutput=False)
    boxes = nc.declare_dram_parameter("boxes", [P, 4], F32, isOutput=False)
    grid = nc.declare_dram_parameter("grid", [P, 2 * NPT], F32, isOutput=False)
    if feat == "int8" and ostore == "bf16":
        scl = nc.declare_dram_parameter("scl", [P, 1], F32, isOutput=False)
    if blend == "pe":
        eye = nc.declare_dram_parameter("eye", [P, P], BF16, isOutput=False)
    odt = I8 if ostore == "int8" else BF16
    out = nc.declare_dram_parameter("out", [P, NPT * C], odt, isOutput=True)

    with tile.TileContext(nc) as tc:
        with (
            tc.tile_pool(name="const", bufs=1) as cpool,
            tc.tile_pool(name="apool", bufs=abufs) as apool,
            tc.tile_pool(name="dpool", bufs=dbufs) as dpool,
            tc.tile_pool(name="work", bufs=bufs) as wpool,
            tc.tile_pool(name="psum", bufs=pbufs, space="PSUM") as ppool,
        ):
            bx = cpool.tile([P, 4], F32)
            nc.sync.dma_start(out=bx[:], in_=boxes[:])
            g = cpool.tile([P, 2 * NPT], F32)
            nc.sync.dma_start(out=g[:], in_=grid[:])
            if feat == "int8" and ostore == "bf16":
                sc = cpool.tile([P, 1], F32)
                nc.sync.dma_start(out=sc[:], in_=scl[:])
            if blend == "pe":
                eye_t = cpool.tile([P, P], BF16)
                nc.sync.dma_start(out=eye_t[:], in_=eye[:])
            BY = g[:, 0:NPT]
            BX = g[:, NPT:2 * NPT]

            xc, yc = bx[:, 0:1], bx[:, 1:2]
            bw, bh = bx[:, 2:3], bx[:, 3:4]

            # per-box scale/translate: yf = BY*(0.5*bh-0.5) + (yc-1)
            sy = cpool.tile([P, 1], F32)
            nc.vector.tensor_scalar(out=sy[:], in0=bh, scalar1=0.5, scalar2=-0.5,
                                    op0=Alu.mult, op1=Alu.add)
            sx = cpool.tile([P, 1], F32)
            nc.vector.tensor_scalar(out=sx[:], in0=bw, scalar1=0.5, scalar2=-0.5,
                                    op0=Alu.mult, op1=Alu.add)
            ty = cpool.tile([P, 1], F32)
            nc.vector.tensor_scalar(out=ty[:], in0=yc, scalar1=-1.0, scalar2=None,
                                    op0=Alu.add)
            tx = cpool.tile([P, 1], F32)
            nc.vector.tensor_scalar(out=tx[:], in0=xc, scalar1=-1.0, scalar2=None,
                                    op0=Alu.add)

            yf = cpool.tile([P, NPT], F32)
            nc.vector.tensor_scalar(out=yf[:], in0=BY, scalar1=sy[:, 0:1],
                                    scalar2=ty[:, 0:1], op0=Alu.mult, op1=Alu.add)
            xf = cpool.tile([P, NPT], F32)
            nc.vector.tensor_scalar(out=xf[:], in0=BX, scalar1=sx[:, 0:1],
                                    scalar2=tx[:, 0:1], op0=Alu.mult, op1=Alu.add)

            def floor_frac(src, nm):
                r = cpool.tile([P, NPT], F32, tag=f"r{nm}")
                nc.vector.tensor_scalar(out=r[:], in0=src[:], scalar1=MAGIC,
                                        scalar2=-MAGIC, op0=Alu.add, op1=Alu.add)
                m = cpool.tile([P, NPT], F32, tag=f"m{nm}")
                nc.vector.tensor_tensor(out=m[:], in0=r[:], in1=src[:], op=Alu.is_gt)
                fl = cpool.tile([P, NPT], F32, tag=f"f{nm}")
                nc.vector.tensor_tensor(out=fl[:], in0=r[:], in1=m[:], op=Alu.subtract)
                fr = cpool.tile([P, NPT], F32, tag=f"w{nm}")
                nc.vector.tensor_tensor(out=fr[:], in0=src[:], in1=fl[:], op=Alu.subtract)
                return fl, fr

            y0, wy = floor_frac(yf[:], "y")
            x0, wx = floor_frac(xf[:], "x")

            wyc = cpool.tile([P, NPT], F32)
            nc.vector.tensor_scalar(out=wyc[:], in0=wy[:], scalar1=-1.0, scalar2=1.0,
                                    op0=Alu.mult, op1=Alu.add)
            wxc = cpool.tile([P, NPT], F32)
            nc.vector.tensor_scalar(out=wxc[:], in0=wx[:], scalar1=-1.0, scalar2=1.0,
                                    op0=Alu.mult, op1=Alu.add)

            wA0 = cpool.tile([P, NPT], F32)
            nc.vector.tensor_tensor(out=wA0[:], in0=wyc[:], in1=wxc[:], op=Alu.mult)
            wA1 = cpool.tile([P, NPT], F32)
            nc.vector.tensor_tensor(out=wA1[:], in0=wyc[:], in1=wx[:], op=Alu.mult)
            wB0 = cpool.tile([P, NPT], F32)
            nc.vector.tensor_tensor(out=wB0[:], in0=wy[:], in1=wxc[:], op=Alu.mult)
            wB1 = cpool.tile([P, NPT], F32)
            nc.vector.tensor_tensor(out=wB1[:], in0=wy[:], in1=wx[:], op=Alu.mult)

            wts = [wA0, wA1, wB0, wB1]
            if feat == "int8" and ostore == "bf16":
                # fold the dequant scale into the blend weights
                for wt in wts:
                    nc.vector.tensor_scalar(out=wt[:], in0=wt[:],
                                            scalar1=sc[:, 0:1], scalar2=None,
                                            op0=Alu.mult)

            # quad index: idx = (clamp(y0,-2,128)+2)*NQ + clamp(x0,-2,128)+2
            cy = cpool.tile([P, NPT], F32)
            nc.vector.tensor_scalar(out=cy[:], in0=y0[:], scalar1=-2.0, scalar2=128.0,
                                    op0=Alu.max, op1=Alu.min)
            cx = cpool.tile([P, NPT], F32)
            nc.vector.tensor_scalar(out=cx[:], in0=x0[:], scalar1=-2.0, scalar2=128.0,
                                    op0=Alu.max, op1=Alu.min)
            aff = cpool.tile([P, NPT], F32)
            nc.vector.tensor_scalar(out=aff[:], in0=cy[:], scalar1=float(NQ),
                                    scalar2=float(2 * NQ + 2), op0=Alu.mult,
                                    op1=Alu.add)
            nc.vector.tensor_tensor(out=aff[:], in0=aff[:], in1=cx[:], op=Alu.add)

            idx = cpool.tile([P, NPT], I32)
            nc.vector.tensor_copy(out=idx[:], in_=aff[:])

            import concourse.bass as _b

            sg = store_group
            assert NPT % sg == 0
            dgs = None
            if diag_once and blend == "pe":
                dgs = []
                for t in range(NPT):
                    dgt = cpool.tile([P, 4, P], BF16, tag=f"dg{t}")
                    for j in range(4):
                        nc.vector.tensor_scalar(
                            out=dgt[:, j], in0=eye_t[:],
                            scalar1=wts[j][:, t:t + 1], scalar2=None,
                            op0=Alu.mult)
                    dgs.append(dgt)
            gq0 = None
            if skip_gather:
                gq0 = cpool.tile([P, 4 * C], BF16, tag="gq0")
                nc.gpsimd.indirect_dma_start(
                    out=gq0[:], out_offset=None, in_=qmap[:],
                    in_offset=_b.IndirectOffsetOnAxis(ap=idx[:, 0:1], axis=0))
            for rep in range(repeats):
                for g_i in range(NPT // sg):
                    afat = apool.tile([P, sg * C], odt, tag="afat")
                    for k in range(sg):
                        t = g_i * sg + k
                        if skip_gather:
                            gq = gq0
                        elif half_gather and k % 2 == 1:
                            gq = prev_gq
                        else:
                            gq = wpool.tile([P, 4 * C], BF16, tag="gq")
                            nc.gpsimd.indirect_dma_start(
                                out=gq[:], out_offset=None, in_=qmap[:],
                                in_offset=_b.IndirectOffsetOnAxis(
                                    ap=idx[:, t:t + 1], axis=0))
                            prev_gq = gq
                        if skip_blend:
                            # tiny per-gather consumer: paces the pipeline
                            # like the real blend without engine cost
                            nc.vector.tensor_copy(out=afat[:, k * 4:k * 4 + 4],
                                                  in_=gq[:, 0:4])
                            continue
                        if blend == "pe":
                            if diag_once:
                                dg = dgs[t]
                            else:
                                dg = dpool.tile([P, 4, P], BF16, tag="dg")
                                for j in range(4):
                                    nc.vector.tensor_scalar(
                                        out=dg[:, j], in0=eye_t[:],
                                        scalar1=wts[j][:, t:t + 1], scalar2=None,
                                        op0=Alu.mult)
                            if k % pb == 0:
                                ps = ppool.tile([P, pb, C], mybir.dt.float32,
                                                tag="ps")
                            nmm = 1 if one_mm else 4
                            for j in range(nmm):
                                nc.tensor.matmul(
                                    ps[:, k % pb], dg[:, j],
                                    gq[:, j * C:(j + 1) * C],
                                    start=(j == 0), stop=(j == nmm - 1))
                            if k % pb == pb - 1 or k == sg - 1:
                                kk = k - (k % pb)
                                if dvm and t % dvm == dvm - 1:
                                    nc.vector.tensor_copy(
                                        out=afat[:, kk * C:(k + 1) * C],
                                        in_=ps[:, 0:(k % pb) + 1])
                                else:
                                    nc.scalar.copy(
                                        out=afat[:, kk * C:(k + 1) * C],
                                        in_=ps[:, 0:(k % pb) + 1])
                                    if act2:
                                        scr = wpool.tile([P, C], odt,
                                                         tag="scr")
                                        nc.scalar.copy(out=scr[:],
                                                       in_=ps[:, k % pb])
                        else:
                            acc = wpool.tile([P, C], F32, tag="acc")
                            nc.vector.tensor_scalar(
                                out=acc[:], in0=gq[:, 0:C],
                                scalar1=wA0[:, t:t + 1],
                                scalar2=None, op0=Alu.mult)
                            nc.vector.scalar_tensor_tensor(
                                out=acc[:], in0=gq[:, C:2 * C],
                                scalar=wA1[:, t:t + 1], in1=acc[:],
                                op0=Alu.mult, op1=Alu.add)
                            nc.vector.scalar_tensor_tensor(
                                out=acc[:], in0=gq[:, 2 * C:3 * C],
                                scalar=wB0[:, t:t + 1], in1=acc[:],
                                op0=Alu.mult, op1=Alu.add)
                            nc.vector.scalar_tensor_tensor(
                                out=afat[:, k * C:(k + 1) * C],
                                in0=gq[:, 3 * C:4 * C],
                                scalar=wB1[:, t:t + 1], in1=acc[:],
                                op0=Alu.mult, op1=Alu.add)
                    nc.sync.dma_start(
                        out=out[:, g_i * sg * C:(g_i + 1) * sg * C],
                        in_=afat[:])

    nc.compile()
    return nc


def _grid_const():
    base = np.linspace(-1.0, 1.0, 7).astype(np.float32)
    by = np.repeat(base, 7)
    bxx = np.tile(base, 7)
    g = np.concatenate([by, bxx])[None, :]
    return np.ascontiguousarray(np.broadcast_to(g, (P, 2 * NPT)).astype(np.float32))


def _quad_features(features, feat=None):
    feat = feat or FEAT
    f = np.zeros((HP, WP, C), dtype=np.float32)
    f[2:130, 2:130] = features
    if feat == "int8":
        s = float(np.max(np.abs(features))) / 127.0
        if s == 0.0:
            s = 1.0
        fb = np.clip(np.rint(f / s), -127, 127).astype(np.int8)
    else:
        s = None
        fb = f.astype(ml_dtypes.bfloat16)
    q = np.concatenate(
        [fb[0:NQ, 0:NQ], fb[0:NQ, 1:NQ + 1], fb[1:NQ + 1, 0:NQ],
         fb[1:NQ + 1, 1:NQ + 1]], axis=2)
    return np.ascontiguousarray(q).reshape(NROW, 4 * C), s


def _core_inputs(qmap, boxes, k, s=None):
    if "grid" not in _STATE:
        _STATE["grid"] = _grid_const()
        _STATE["eye"] = np.eye(P, dtype=ml_dtypes.bfloat16)
    m = {
        "qmap": qmap,
        "boxes": np.ascontiguousarray(boxes[k * P:(k + 1) * P]),
        "grid": _STATE["grid"],
    }
    if BLEND == "pe":
        m["eye"] = _STATE["eye"]
    if s is not None and OSTORE == "bf16":
        m["scl"] = np.full((P, 1), s, dtype=np.float32)
    return m


def kernel(features, boxes, image_height=128, image_width=128):
    from concourse.bass_utils import run_bass_kernel_spmd

    if "nc" not in _STATE:
        _STATE["nc"] = _build_nc()
    nc = _STATE["nc"]

    qmap, s = _quad_features(np.asarray(features, dtype=np.float32))
    boxes = np.asarray(boxes, dtype=np.float32)
    in_maps = [_core_inputs(qmap, boxes, k, s) for k in range(NCORES)]
    res = run_bass_kernel_spmd(
        nc, in_maps, core_ids=list(range(NCORES)),
        trace=_STATE.get("trace", False),
    )
    _STATE["last"] = res
    out = np.concatenate(
        [res.results[k]["out"].astype(np.float32).reshape(P, 7, 7, C)
         for k in range(NCORES)], axis=0
    )
    if OSTORE == "int8":
        out *= np.float32(s)  # dequantize: device stored round(sum(w*q))
    return out
